# revision 1
# baseline (speedup 1.0000x reference)
"""Trainium2 Bass kernel for the CO2-electrolysis surrogate model.

Contract: kernel(**inputs) takes FULL unsharded inputs (x [16384,5], MLP
weights, kinetic params i0/alpha) and returns the FULL [16384,2] output.
Internally: batch is sharded 2048-per-core across 8 NeuronCores (pure data
parallel); i0/alpha (6 scalars) are baked into the program as immediates;
weights are replicated runtime inputs.

Algorithm notes: i_tot(g) is strictly decreasing in the voltage-grid index g,
so argmin_g |i_tot(g) - I_TARGET| is found with a branchless 5-step 4-ary
climb (3 probes/step; b = last g with i_tot >= target over a virtual 1024
grid) followed by an exact batched 2-point refinement, instead of evaluating
all 1000 grid points. Probe exponentials reuse one ACT exp via constant
multiplies exp(sc*s), exp(2*sc*s) baked as inline tensors.
"""

import sys

for _p in ("/opt/trn_rl_repo", "/opt/pypackages"):
    if _p not in sys.path:
        sys.path.insert(0, _p)

from contextlib import ExitStack

import numpy as np

import concourse.bacc as bacc
import concourse.bass as bass
import concourse.tile as tile
from concourse import mybir
from concourse._compat import with_exitstack

F32 = mybir.dt.float32
AF = mybir.ActivationFunctionType
OP = mybir.AluOpType

# ---- problem constants (match reference.py) ----
N = 16384
NCORES = 8
NPC = N // NCORES            # 2048 samples per core
NT = NPC // 128              # 16 tiles of 128 samples
HID = 64
GRID = 1000
VMIN, VMAX = -1.25, 0.0
I_TARGET = 200.0
F_CONST = 96485.33
RT = 8.314 * 298.15
D_CO2 = 1.91e-9
C_CO2 = 34.0
E_EQ = (-0.11, 0.08, 0.0)
N_ELEC_CO2 = (2.0, 12.0)
DV = (VMAX - VMIN) / (GRID - 1)
_DBG_STAGE = 0
FRT = F_CONST / RT


@with_exitstack
def _body(ctx, tc, io, i0, alpha):
    nc = tc.nc
    x_d, W1_d, pack_d, b4_d, out_d = io

    # per-species immediates
    sc = [float(alpha[k] * FRT * DV) for k in range(3)]                 # d(arg)/dg
    t0 = [float(alpha[k] * FRT * (VMIN - E_EQ[k])) for k in range(3)]   # arg at g=0
    # (N_ELEC * F) rounded f32 like the reference's constant folding
    c1 = [float(np.float32(np.float32(N_ELEC_CO2[k]) * F_CONST)) for k in range(2)]

    singles = ctx.enter_context(tc.tile_pool(name="singles", bufs=1))
    work = ctx.enter_context(tc.tile_pool(name="work", bufs=3))
    psum = ctx.enter_context(tc.tile_pool(name="psum", bufs=8, space="PSUM"))

    # dummy first ACT op: makes the initial table load pick
    # sigmoid_and_friends (covers copy+relu+sigmoid -> one load fewer)
    warm = singles.tile([128, 1], F32)
    nc.vector.memset(warm, 0.0)
    nc.scalar.activation(warm, warm, AF.Sigmoid, scale=1.0)

    # ---------- weights / constants into SBUF ----------
    # single inline-const blob: [eye128 | per-step probe exp factors]
    STEPS = [256, 64, 16, 4, 1]
    csz = len(STEPS) * 2 * 3 * NT
    if not hasattr(nc, "_constblob"):
        blob = np.empty((128, 128 + csz), np.float32)
        blob[:, 0:128] = np.eye(128, dtype=np.float32)
        fdat = np.empty((128, len(STEPS), 2, 3, NT), np.float32)
        for jj, ss in enumerate(STEPS):
            for k in range(3):
                fdat[:, jj, 0, k, :] = np.float32(np.exp(sc[k] * ss))
                fdat[:, jj, 1, k, :] = np.float32(np.exp(2.0 * sc[k] * ss))
        blob[:, 128:] = fdat.reshape(128, -1)
        nc._constblob = nc.inline_tensor(blob, name="constblob")
    constsb = singles.tile([128, 128 + csz], F32)
    nc.sync.dma_start(constsb, nc._constblob.ap())
    ident = constsb[:, 0:128]
    fsb = constsb[:, 128:].rearrange("p (j w k t) -> p j w k t", j=len(STEPS), w=2, k=3)

    # All matmuls/transposes keep PE tile position (0,0): operands at
    # partition base 0 everywhere (mixed row-bases sharing a column base
    # crash the device).
    W1sb = singles.tile([5, 64], F32)
    nc.sync.dma_start(W1sb, W1_d)
    # pack_d: host-packed [64, 137] = [W2 | W3 | W4 | b1 | b2 | b3]
    packsb = singles.tile([64, 137], F32)
    nc.sync.dma_start(packsb, pack_d)
    W2sb = packsb[:, 0:64]
    W3sb = packsb[:, 64:128]
    W4sb = packsb[:, 128:134]
    b1sb = packsb[:, 134:135]
    b2sb = packsb[:, 135:136]
    b3sb = packsb[:, 136:137]
    b4sb = singles.tile([128, 6], F32)
    nc.sync.dma_start(
        b4sb, bass.AP(tensor=b4_d.tensor, offset=b4_d.offset, ap=[[0, 128], [1, 6]])
    )

    # ---------- load x ----------
    # x dram [2048, 5] -> sbuf [128p, 16t, 5k]; sample s = t*128 + p
    xsb = singles.tile([128, NT, 5], F32)
    nc.sync.dma_start(xsb, x_d.rearrange("(p t) k -> p t k", t=NT))

    if _DBG_STAGE == 10:   # debug: x load only
        o = singles.tile([128, NT, 2], F32)
        nc.vector.tensor_copy(o, xsb[:, :, 0:2])
        nc.gpsimd.dma_start(out_d.rearrange("(p t) c -> p t c", t=NT), o)
        return

    # ---------- MLP (activations live as [64 hid, 2048 samples]) ----------
    # 16 transposes: xT[k, t, p] = x[t*128+p, k]
    xT_ps = [psum.tile([128, 512], F32, tag="ps", name=f"xTps{i}") for i in range(4)]
    for t in range(NT):
        nc.tensor.transpose(
            xT_ps[t // 4][0:5, (t % 4) * 128:(t % 4 + 1) * 128], xsb[:, t, :], ident
        )
    xT = singles.tile([5, NT, 128], F32)
    for i in range(4):
        src = xT_ps[i][0:5, :].rearrange("p (a c) -> p a c", a=4)
        dst = xT[:, 4 * i:4 * i + 4, :]
        if i % 2:
            nc.scalar.copy(dst, src)
        else:
            nc.vector.tensor_copy(dst, src)

    def relu_from(ps_tiles, bias, name):
        h = work.tile([64, 4, 512], F32, tag=name, name=name)
        for i in range(4):
            if i % 2:
                nc.scalar.activation(
                    h[:, i, :], ps_tiles[i][0:64, :], AF.Relu, bias=bias, scale=1.0
                )
            else:
                nc.vector.tensor_scalar(
                    h[:, i, :], ps_tiles[i][0:64, :], bias, 0.0, OP.add, OP.max
                )
        return h.rearrange("p a c -> p (a c)")   # [64, 2048]

    if _DBG_STAGE == 11:   # debug: through transposes+copies
        o = singles.tile([128, NT, 2], F32)
        nc.vector.tensor_copy(o[0:5], xT[:, :, 0:2])
        nc.gpsimd.dma_start(out_d.rearrange("(p t) c -> p t c", t=NT), o)
        return

    # layer 1: 4 block matmuls K=5, M=64, N=512; A-chunks (t<8) land in psum
    # rows 0:64 (tile pos (0,0)), B-chunks in rows 64:128 (pos (0,64)) -- row
    # base never changes, only the out column, which is the safe transition.
    h1ps = [psum.tile([128, 512], F32, tag="ps", name=f"h1ps{i}") for i in range(4)]
    for i in range(4):
        nc.tensor.matmul(h1ps[i][0:64, :], W1sb, xT[:, 4 * i:4 * i + 4, :])
    h1 = relu_from(h1ps, b1sb, "h1")
    if _DBG_STAGE == 12:   # debug: dump h1 slice
        nc.gpsimd.dma_start(
            out_d.rearrange("(p a) c -> p a c", p=64),
            h1[0:64, 0:64].rearrange("p (a c) -> p a c", c=2))
        return

    h2ps = [psum.tile([128, 512], F32, tag="ps", name=f"h2ps{i}") for i in range(4)]
    for i in range(4):
        nc.tensor.matmul(h2ps[i][0:64, :], W2sb, h1[:, i * 512:(i + 1) * 512])
    h2 = relu_from(h2ps, b2sb, "h2")
    h3ps = [psum.tile([128, 512], F32, tag="ps", name=f"h3ps{i}") for i in range(4)]
    for i in range(4):
        nc.tensor.matmul(h3ps[i][0:64, :], W3sb, h2[:, i * 512:(i + 1) * 512])
    h3 = relu_from(h3ps, b3sb, "h3")

    # layer 4: back to samples-on-partitions: lat[p, t, j]
    latps = psum.tile([128, 96], F32, tag="ps")
    for t in range(NT):
        nc.tensor.matmul(
            latps[:, t * 6:(t + 1) * 6],
            h3[:, t * 128:(t + 1) * 128],
            W4sb,
        )
    lat = singles.tile([128, NT, 6], F32)
    nc.vector.tensor_tensor(
        lat,
        latps.rearrange("p (t j) -> p t j", j=6),
        bass.AP(tensor=b4sb.tensor, offset=b4sb.offset,
                ap=[list(b4sb[:].ap[0]), [0, NT], [1, 6]]),
        OP.add,
    )

    if _DBG_STAGE == 1:   # debug: dump lat into out and stop
        nc.gpsimd.dma_start(
            out_d.rearrange("(t p) c -> p t c", p=128), lat[:, :, 0:2]
        )
        return

    def latj(j):
        return lat[:, :, j]

    # ---------- per-sample derived params ----------
    def t16(name):
        return singles.tile([128, NT], F32, name=name)

    (r, eps, one_m, omr, L, absL, Kdl, seps, Deff, Dinv, KD, Kgdl,
     Kginv, LD, ssum, Ktot, rinv, mm, st, rst) = (
        t16(n) for n in ("r", "eps", "one_m", "omr", "L", "absL", "Kdl", "seps",
                         "Deff", "Dinv", "KD", "Kgdl", "Kginv", "LD", "ssum",
                         "Ktot", "rinv", "mm", "st", "rst"))

    nc.scalar.activation(eps, latj(1), AF.Sigmoid, scale=1.0)
    nc.scalar.activation(seps, eps, AF.Sqrt, scale=1.0)
    # gate = 0, but data-dependent on seps: forces every Exp after Sqrt so
    # the scheduler emits exactly one sigmoid->sqrt->exp table-load sequence
    gate = singles.tile([128, 1], F32)
    nc.vector.tensor_scalar_mul(gate, seps[:, 0:1], 0.0)
    lat0g = singles.tile([128, NT], F32)
    lat2g = singles.tile([128, NT], F32)
    nc.vector.tensor_scalar(lat0g, latj(0), gate, None, OP.add)
    nc.vector.tensor_scalar(lat2g, latj(2), gate, None, OP.add)
    nc.scalar.activation(r, lat0g, AF.Exp, scale=1.0)            # exp(lat0)
    nc.scalar.activation(Kdl, lat2g, AF.Exp, scale=1.0)
    nc.vector.tensor_scalar_mul(r, r, 4e-8)                      # r
    nc.vector.reciprocal(rinv, r)
    nc.vector.tensor_scalar(one_m, eps, -1.0, -1.0, OP.add, OP.mult)   # 1-eps
    nc.vector.reciprocal(omr, one_m)
    nc.vector.tensor_tensor(L, xsb[:, :, 3], omr, OP.mult)       # zlt/(1-eps)
    nc.vector.tensor_tensor(Deff, eps, seps, OP.mult)            # eps^1.5
    nc.vector.tensor_scalar_mul(Deff, Deff, D_CO2)
    nc.vector.reciprocal(Dinv, Deff)
    nc.vector.tensor_tensor(KD, Kdl, Deff, OP.mult)
    nc.vector.tensor_tensor(Kgdl, KD, rinv, OP.mult)
    nc.vector.reciprocal(Kginv, Kgdl)
    # |L|/Deff = max(L*Dinv, -L*Dinv)  (avoids the ACT Abs table load)
    nc.vector.tensor_tensor(LD, L, Dinv, OP.mult)
    nc.vector.tensor_scalar_mul(absL, LD, -1.0)
    nc.vector.tensor_tensor(LD, LD, absL, OP.max)
    nc.vector.tensor_tensor(ssum, Kginv, LD, OP.add)
    nc.vector.reciprocal(Ktot, ssum)
    C_all = singles.tile([128, 3, NT], F32)     # 1/i_lim per species (k-major)
    for k in range(2):
        ilim = work.tile([128, NT], F32, tag="w16")
        nc.vector.tensor_scalar(ilim, Ktot, c1[k], float(C_CO2), OP.mult, OP.mult)
        nc.vector.reciprocal(C_all[:, k, :], ilim)
    nc.vector.memset(C_all[:, 2, :], 0.0)       # H2 not transport limited

    # softmax(2*lat[3:6]) -> thetas; A_k = 1/(theta_k * i0_k)
    nc.vector.tensor_tensor(mm, latj(3), latj(4), OP.max)
    nc.vector.tensor_tensor(mm, mm, latj(5), OP.max)
    nc.vector.tensor_scalar(mm, mm, gate, None, OP.add)
    T_all = singles.tile([128, 3, NT], F32)
    for k in range(3):
        d = work.tile([128, NT], F32, tag="w16")
        nc.vector.tensor_tensor(d, latj(3 + k), mm, OP.subtract)
        nc.scalar.activation(T_all[:, k, :], d, AF.Exp, scale=2.0)
    nc.vector.tensor_tensor(st, T_all[:, 0, :], T_all[:, 1, :], OP.add)
    nc.vector.tensor_tensor(st, st, T_all[:, 2, :], OP.add)
    nc.vector.reciprocal(rst, st)
    Ti_all = singles.tile([128, 3, NT], F32)    # theta_k * i0_k
    for k in range(3):
        nc.vector.scalar_tensor_tensor(
            Ti_all[:, k, :], T_all[:, k, :], float(i0[k]), rst, OP.mult, OP.mult
        )
    A_all = singles.tile([128, 3, NT], F32)
    nc.vector.reciprocal(A_all, Ti_all)

    if _DBG_STAGE == 2:   # debug: dump Ktot/theta-derived A into out and stop
        nc.gpsimd.dma_start(
            out_d.rearrange("(t p) c -> p t c", p=128)[:, :, 0], Ktot)
        return

    # ---------- binary climb: b = last g in virtual grid with i_tot >= target ----------
    b = singles.tile([128, NT], F32)
    nc.vector.memset(b, -1.0)
    nc.vector.tensor_scalar(b, b, gate, None, OP.add)   # order search after sqrt

    def eval_itot(g_ap, S, itot, step_ofs=0.0):
        """S[p,3,t] = i_sp at grid idx (g + step_ofs); itot[p,t] = sum_k S."""
        arg = work.tile([128, 3, NT], F32, tag="arg")
        for k in range(3):
            nc.vector.tensor_scalar(
                arg[:, k, :], g_ap, sc[k], t0[k] + sc[k] * step_ofs, OP.mult, OP.add
            )
        E = work.tile([128, 3, NT], F32, tag="E")
        nc.scalar.activation(E, arg, AF.Exp, scale=1.0)
        nc.vector.tensor_tensor(E, E, A_all, OP.mult)     # A*invE
        nc.vector.tensor_tensor(E, E, C_all, OP.add)      # + 1/i_lim
        nc.vector.reciprocal(S, E)                        # i_sp
        nc.vector.reduce_sum(itot, S.rearrange("p k t -> p t k"), axis=mybir.AxisListType.X)

    # 4-ary climb: 5 steps, 3 probes each (s = 256,64,16,4,1 covers [-1,1022])
    # probe j's exp factor exp(sc_k * j * s) applied as a const multiply.

    for j, s in enumerate(STEPS):
        s = float(s)
        E = work.tile([128, 3, 3, NT], F32, tag="E")    # [p, probe, species, t]
        arg = work.tile([128, 3, NT], F32, tag="arg")
        for k in range(3):
            nc.vector.tensor_scalar(
                arg[:, k, :], b, sc[k], t0[k] + sc[k] * s, OP.mult, OP.add
            )
        nc.scalar.activation(E[:, 0], arg, AF.Exp, scale=1.0)
        nc.gpsimd.tensor_tensor(E[:, 1], E[:, 0], fsb[:, j, 0], OP.mult)
        nc.vector.tensor_tensor(E[:, 2], E[:, 0], fsb[:, j, 1], OP.mult)
        P = work.tile([128, 3, 3, NT], F32, tag="P")
        nc.vector.tensor_tensor(P[:, 0], E[:, 0], A_all, OP.mult)
        nc.gpsimd.tensor_tensor(P[:, 1], E[:, 1], A_all, OP.mult)
        nc.vector.tensor_tensor(P[:, 2], E[:, 2], A_all, OP.mult)
        nc.vector.tensor_tensor(P[:, 0], P[:, 0], C_all, OP.add)
        nc.gpsimd.tensor_tensor(P[:, 1], P[:, 1], C_all, OP.add)
        nc.vector.tensor_tensor(P[:, 2], P[:, 2], C_all, OP.add)
        S = work.tile([128, 3, 3, NT], F32, tag="S")
        nc.vector.reciprocal(S, P)
        itot = work.tile([128, 3, NT], F32, tag="it")
        nc.vector.reduce_sum(
            itot, S.rearrange("p j k t -> p j t k"), axis=mybir.AxisListType.X
        )
        pred = work.tile([128, 3, NT], F32, tag="pd")
        nc.vector.tensor_scalar(pred, itot, I_TARGET, s, OP.is_ge, OP.mult)
        cnt = work.tile([128, NT], F32, tag="ct")
        nc.vector.reduce_sum(
            cnt, pred.rearrange("p j t -> p t j"), axis=mybir.AxisListType.X
        )
        nc.vector.tensor_tensor(b, b, cnt, OP.add)

    if _DBG_STAGE == 3:   # debug: dump b and stop
        nc.gpsimd.dma_start(
            out_d.rearrange("(t p) c -> p t c", p=128)[:, :, 0], b)
        return

    # ---------- refine: evaluate the two bracketing real-grid points ----------
    g0 = singles.tile([128, NT], F32)
    g1 = singles.tile([128, NT], F32)
    nc.vector.tensor_scalar(g0, b, 0.0, float(GRID - 1), OP.max, OP.min)
    nc.vector.tensor_scalar(g1, b, 1.0, float(GRID - 1), OP.add, OP.min)

    # one batched eval of both bracketing points: [p, point, species, t]
    SP = singles.tile([128, 2, 3, NT], F32, name="SP")
    argP = singles.tile([128, 2, 3, NT], F32, name="argP")
    for gi, g_ap in enumerate((g0, g1)):
        for k in range(3):
            nc.vector.tensor_scalar(
                argP[:, gi, k, :], g_ap, sc[k], t0[k], OP.mult, OP.add
            )
    nc.scalar.activation(argP, argP, AF.Exp, scale=1.0)
    nc.vector.tensor_tensor(argP[:, 0], argP[:, 0], A_all, OP.mult)
    nc.gpsimd.tensor_tensor(argP[:, 1], argP[:, 1], A_all, OP.mult)
    nc.vector.tensor_tensor(argP[:, 0], argP[:, 0], C_all, OP.add)
    nc.gpsimd.tensor_tensor(argP[:, 1], argP[:, 1], C_all, OP.add)
    nc.vector.reciprocal(SP, argP)
    itP = singles.tile([128, 2, NT], F32, name="itP")
    nc.vector.reduce_sum(
        itP, SP.rearrange("p g k t -> p g t k"), axis=mybir.AxisListType.X
    )
    d0 = singles.tile([128, NT], F32, name="d0")
    # d0 <= d1  <=>  it0 + it1 <= 2*target  (it0 >= it1, monotone decreasing)
    nc.vector.tensor_tensor(d0, itP[:, 0, :], itP[:, 1, :], OP.add)
    pick0 = singles.tile([128, NT], mybir.dt.int32)
    nc.vector.tensor_scalar(pick0, d0, 2.0 * I_TARGET, None, OP.is_le)
    Ssel = singles.tile([128, 3, NT], F32)
    for k in range(3):
        nc.vector.select(Ssel[:, k, :], pick0, SP[:, 0, k, :], SP[:, 1, k, :])

    tot = singles.tile([128, NT], F32)
    nc.vector.reduce_sum(tot, Ssel.rearrange("p k t -> p t k"), axis=mybir.AxisListType.X)
    rtot = singles.tile([128, NT], F32)
    nc.vector.reciprocal(rtot, tot)
    fe_out = singles.tile([128, NT, 2], F32)
    nc.vector.tensor_tensor(fe_out[:, :, 0], Ssel[:, 1, :], rtot, OP.mult)  # FE_C2H4
    nc.vector.tensor_tensor(fe_out[:, :, 1], Ssel[:, 0, :], rtot, OP.mult)  # FE_CO

    nc.gpsimd.dma_start(out_d.rearrange("(p t) c -> p t c", t=NT), fe_out)


def _build(i0, alpha, reps=1):
    nc = bacc.Bacc("TRN2", target_bir_lowering=False, debug=False)
    x_d = nc.dram_tensor("x", [NPC, 5], F32, kind="ExternalInput").ap()
    W1_d = nc.dram_tensor("W1", [5, HID], F32, kind="ExternalInput").ap()
    pack_d = nc.dram_tensor("pack", [HID, 137], F32, kind="ExternalInput").ap()
    b4_d = nc.dram_tensor("b4", [6], F32, kind="ExternalInput").ap()
    out_d = nc.dram_tensor("out", [NPC, 2], F32, kind="ExternalOutput").ap()
    io = (x_d, W1_d, pack_d, b4_d, out_d)
    with tile.TileContext(nc) as tc:
        for _ in range(reps):
            _body(tc, io, np.asarray(i0, np.float64), np.asarray(alpha, np.float64))
    nc.compile()
    return nc


_CACHE = {}


def kernel(x, W1, b1, W2, b2, W3, b3, W4, b4, i0, alpha):
    from concourse.bass_utils import run_bass_kernel_spmd

    x = np.ascontiguousarray(np.asarray(x, np.float32))
    i0 = np.asarray(i0, np.float32)
    alpha = np.asarray(alpha, np.float32)
    key = (i0.tobytes(), alpha.tobytes())
    if key not in _CACHE:
        _CACHE[key] = _build(i0, alpha)
    nc = _CACHE[key]

    pack = np.concatenate(
        [np.asarray(W2, np.float32), np.asarray(W3, np.float32),
         np.asarray(W4, np.float32), np.asarray(b1, np.float32)[:, None],
         np.asarray(b2, np.float32)[:, None], np.asarray(b3, np.float32)[:, None]],
        axis=1)
    common = {
        "W1": np.ascontiguousarray(W1, np.float32),
        "pack": np.ascontiguousarray(pack),
        "b4": np.ascontiguousarray(b4, np.float32),
    }
    in_maps = [
        {"x": x[c * NPC:(c + 1) * NPC], **common} for c in range(NCORES)
    ]
    res = run_bass_kernel_spmd(nc, in_maps, core_ids=list(range(NCORES)))
    return np.concatenate([res.results[c]["out"] for c in range(NCORES)], axis=0)



# revision 49
# speedup vs baseline: 3.2435x; 3.2435x over previous
"""Trainium2 Bass kernel for the CO2-electrolysis surrogate model.

Contract: kernel(**inputs) takes FULL unsharded inputs (x [16384,5], MLP
weights, kinetic params i0/alpha) and returns the FULL [16384,2] output.
Internally: batch is sharded 2048-per-core across 8 NeuronCores (pure data
parallel).

Design (v4):
- x is transposed on the host into xT [5, 2048] (column c = t*128+p holds
  sample s = p*16+t) so the MLP needs no PE transposes. zlt (= x[:,3]), the
  search/kinetics constant table (functions of i0/alpha), and b4 ride in one
  packed [128, 91] input; 4 input DMAs per rep.
- Matmul inputs are float32r end-to-end (1 cycle/row at N=512 vs 4 for fp32).
- Relu reads two-bank PSUM tiles [128, 2x512], split DVE/ACT (tensor_scalar
  is not implemented on the Pool engine).
- Reps are emitted in groups of G: the MLPs run per rep (interleaved
  emission), then ONE grouped tail runs the per-sample physics for all G
  reps with a leading rep axis in every tile - 3x fewer instructions and
  semaphore hops on the serial search chain.
- The voltage search keeps multiplicative state AE[p,r,t,k] = 1/i_kin at the
  current virtual-grid index b; each 4-ary step probes b+{1,2,3}*s via baked
  factor multiplies and updates AE *= exp(sc*s*u) with u = #successful
  probes (no exp of per-sample args; b stays off the critical path).
- The parameter section uses only {exp, ln, abs, relu, copy}, all inside
  activation-table set 6 (natural_log_exp_and_others), loaded once.
  Identities: 1/(1-sigmoid(l)) = 1+e^l, sigmoid(l)^-1.5 = exp(1.5*ln(1+e^-l)).
- b <= 998 always (i_tot(999) < 1e-3 << target for any theta <= 1), so the
  refine step only needs the b < 0 boundary case.
"""

import sys

for _p in ("/opt/trn_rl_repo", "/opt/pypackages"):
    if _p not in sys.path:
        sys.path.insert(0, _p)

import math

import numpy as np

import concourse.bacc as bacc
import concourse.bass as bass
import concourse.tile as tile
from concourse import mybir

F32 = mybir.dt.float32
F32R = mybir.dt.float32r
I32 = mybir.dt.int32
AF = mybir.ActivationFunctionType
OP = mybir.AluOpType

# ---- problem constants (match reference.py) ----
N = 16384
NCORES = 8
NPC = N // NCORES            # 2048 samples per core
NT = NPC // 128              # 16 tiles of 128 samples
HID = 64
GRID = 1000
VMIN, VMAX = -1.25, 0.0
I_TARGET = 200.0
F_CONST = 96485.33
RT = 8.314 * 298.15
D_CO2 = 1.91e-9
C_CO2 = 34.0
E_EQ = (-0.11, 0.08, 0.0)
N_ELEC_CO2 = (2.0, 12.0)
DV = (VMAX - VMIN) / (GRID - 1)
FRT = F_CONST / RT
STEPS = [256, 64, 16, 4, 1]   # 4-ary climb over virtual 1024-grid, b in [-1,1022]
GRPSZ = 3                     # reps per grouped tail
_DBG_STAGE = 0

# blobz layout: [zlt(16) | blob(NBLOB) | b4(6)]
BL0 = NT                      # blob base column inside blobz
LNF_C = 45
CAE_C = 60
F1_C = 63
LN4_C = 66
CIL_C = 67
NBLOB = 70


def _make_blob_row(i0, alpha):
    """Search/kinetics constants [NBLOB] f32 (functions of i0/alpha)."""
    i0 = np.asarray(i0, np.float64)
    alpha = np.asarray(alpha, np.float64)
    sc = [float(alpha[k] * FRT * DV) for k in range(3)]
    t0 = [float(alpha[k] * FRT * (VMIN - E_EQ[k])) for k in range(3)]
    cols = []
    for s in STEPS:                 # 0:45  probe factors exp(sc*j*s)
        for j in (1, 2, 3):
            for k in range(3):
                cols.append(np.exp(sc[k] * j * s))
    for s in STEPS:                 # 45:60 ln of climb factor: sc_k * s
        for k in range(3):
            cols.append(sc[k] * s)
    for k in range(3):              # 60:63 cAE: exp(t0-sc)/i0 (AE at b=-1, pre 1/theta)
        cols.append(np.exp(t0[k] - sc[k]) / float(i0[k]))
    for k in range(3):              # 63:66 f1: one-step factor exp(sc)
        cols.append(np.exp(sc[k]))
    cols.append(math.log(4e-8))     # 66    bias for the r/Kdl exp
    # 67:70  1/i_lim prefactors (3rd species: H2, not transport-limited -> 0)
    for nk in N_ELEC_CO2:
        cols.append(1.0 / (float(np.float32(np.float32(nk) * F_CONST))
                           * C_CO2 * D_CO2))
    cols.append(0.0)
    row = np.asarray(cols, np.float32)
    assert row.size == NBLOB
    return row


class _Pools:
    pass


def _mk_pools(ctx, tc):
    p = _Pools()
    p.io = ctx.enter_context(tc.tile_pool(name="io", bufs=3))
    p.work = ctx.enter_context(tc.tile_pool(name="work", bufs=2))
    p.psum = ctx.enter_context(tc.tile_pool(name="psum", bufs=3, space="PSUM"))
    return p


def _mlp(tc, po, io, r, g, lat3, azlt3, shared, first):
    """Generator: per-rep DMAs + MLP; writes lat into lat3[:, r] and
    |zlt| into azlt3[:, r].  shared[0] collects the rep's blobsb tile."""
    nc = tc.nc
    xT_d, W1_d, pack_d, blobz_d, out_d = io

    if first:
        # lock the activation table to set 6 (natural_log_exp_and_others):
        # covers exp/ln/abs/relu/copy -> zero reloads for the whole program
        inst = mybir.InstLoadActFuncSet(
            name=nc.get_next_instruction_name(), act_func_set_id=6, ins=[], outs=[])
        nc.scalar.add_instruction(inst)

    xTsb = po.io.tile([5, 4, 512], F32R, name="xTsb")
    nc.sync.dma_start(xTsb, xT_d.rearrange("k (i n) -> k i n", i=4))
    packsb = po.io.tile([64, 137], F32R, name="packsb")
    nc.sync.dma_start(packsb, pack_d)
    W1sb = po.io.tile([5, 64], F32R, name="W1sb")
    nc.sync.dma_start(W1sb, W1_d)
    blobsb = po.io.tile([128, NT + NBLOB + 6], F32, name="blobsb")
    nc.sync.dma_start(blobsb, blobz_d)
    shared[r] = blobsb

    W2sb = packsb[:, 0:64]
    W3sb = packsb[:, 64:128]
    W4sb = packsb[:, 128:134]
    biases = [packsb[:, 134 + i:135 + i].bitcast(F32) for i in range(3)]

    def layer(W, movsrc, bias, name, eng):
        ps = [po.psum.tile([128, 2, 512], F32, tag="ps2", name=f"{name}ps{i}")
              for i in range(2)]
        for i in range(4):
            nc.tensor.matmul(ps[i // 2][0:64, i % 2, :], W, movsrc(i))
        h = po.work.tile([64, 4, 512], F32R, tag=name, name=name, bufs=3)
        for i in range(2):
            dst = h[:, 2 * i:2 * i + 2, :]
            src = ps[i][0:64, :, :]
            if eng[i] == "v":
                nc.vector.tensor_scalar(dst, src, bias, 0.0, OP.add, OP.max)
            else:
                nc.scalar.activation(dst, src, AF.Relu, bias=bias, scale=1.0)
        return h

    yield
    h1 = layer(W1sb, lambda i: xTsb[:, i, :], biases[0], "h1", "va")
    nc.scalar.activation(azlt3[:, r], blobsb[:, 0:NT], AF.Abs, scale=1.0)
    yield
    h2 = layer(W2sb, lambda i: h1[:, i, :], biases[1], "h2", "av")
    yield
    h3 = layer(W3sb, lambda i: h2[:, i, :], biases[2], "h3", "va")
    yield
    h3f = h3.rearrange("p a c -> p (a c)")

    latps = po.psum.tile([128, 96], F32, tag="lat", name="latps", bufs=2)
    for t in range(NT):
        nc.tensor.matmul(latps[:, t * 6:(t + 1) * 6],
                         h3f[:, t * 128:(t + 1) * 128], W4sb)
    b4b = bass.AP(tensor=blobsb.tensor, offset=blobsb.offset + BL0 + NBLOB,
                  ap=[list(blobsb.ap[0]), [0, NT], [1, 6]])
    nc.vector.tensor_tensor(lat3[:, r], latps.rearrange("p (t j) -> p t j", j=6),
                            b4b, OP.add)


def _tail(tc, po, out_d, g, lat3, azlt3, blobsb):
    """Grouped per-sample physics for g reps: parameters, 4-ary climb,
    refine, FE output.  All tiles carry a leading rep axis of size g."""
    nc = tc.nc
    GNT = g * NT

    def bcol(c, n=3):
        """blob columns c..c+n as [p, r(bcast), t(bcast), k] AP"""
        return bass.AP(tensor=blobsb.tensor, offset=blobsb.offset + BL0 + c,
                       ap=[list(blobsb.ap[0]), [0, g], [0, NT], [1, n]])

    if _DBG_STAGE == 1:
        nc.sync.dma_start(out_d.rearrange("(p t) c -> p t c", t=NT),
                          lat3[:, 0, :, 0:2])
        return

    def w3(name, dt=F32):
        return po.work.tile([128, GRPSZ, NT], dt, tag=name, name=name)

    def w33(name):
        return po.work.tile([128, GRPSZ, NT, 3], F32, tag=name, name=name)

    a1, e1i, a2, a3, d1, a4, Lt, s5, t6, mm, st, b = (
        w3(n) for n in ("a1", "e1i", "a2", "a3", "d1", "a4", "Lt",
                        "s5", "t6", "mm", "st", "b"))
    C_all, dk, T, iT, t7, AE = (
        w33(n) for n in ("C_all", "dk", "T", "iT", "t7", "AE"))
    Cfull = po.work.tile([128, 3, GRPSZ, NT, 3], F32, tag="Cfull", name="Cfull")

    def sl(t):       # restrict leading-rep-axis tiles to g reps
        return t[:, 0:g] if g != GRPSZ else t

    l3 = sl(lat3)
    nc.scalar.activation(sl(a1), l3[:, :, :, 1], AF.Exp, scale=-1.0)   # e^-l1
    nc.vector.reciprocal(sl(e1i), sl(a1))                              # e^l1
    nc.scalar.activation(sl(a2), sl(a1), AF.Ln, bias=1.0, scale=1.0)   # ln(1+e^-l1)
    nc.scalar.activation(sl(a3), sl(a2), AF.Exp, scale=1.5)            # eps^-1.5
    nc.gpsimd.tensor_tensor(sl(d1), l3[:, :, :, 0], l3[:, :, :, 2], OP.subtract)
    nc.scalar.activation(sl(a4), sl(d1), AF.Exp,
                         bias=blobsb[:, BL0 + LN4_C:BL0 + LN4_C + 1],
                         scale=1.0)                                    # r/Kdl
    nc.vector.scalar_tensor_tensor(sl(Lt), sl(e1i), 1.0, sl(azlt3),
                                   OP.add, OP.mult)
    nc.gpsimd.tensor_tensor(sl(s5), sl(a4), sl(Lt), OP.add)
    nc.gpsimd.tensor_tensor(sl(t6), sl(s5), sl(a3), OP.mult)
    # C_all[:, r, :, k] = t6 * cilim_k  (k=2 blob column is 0 -> H2 term)
    t6b = bass.AP(tensor=t6.tensor, offset=t6.offset,
                  ap=[list(t6.ap[0]), [NT, g], [1, NT], [0, 3]])
    nc.vector.tensor_tensor(sl(C_all), t6b, bcol(CIL_C), OP.mult)

    # softmax thetas -> AE init at b=-1
    nc.vector.reduce_max(sl(mm), l3[:, :, :, 3:6], axis=mybir.AxisListType.X,
                         opt_input=False)
    mmb = bass.AP(tensor=mm.tensor, offset=mm.offset,
                  ap=[list(mm.ap[0]), [NT, g], [1, NT], [0, 3]])
    nc.vector.tensor_tensor(sl(dk), l3[:, :, :, 3:6], mmb, OP.subtract)
    nc.scalar.activation(sl(T), sl(dk), AF.Exp, scale=2.0)
    nc.vector.reduce_sum(sl(st), sl(T), axis=mybir.AxisListType.X,
                         opt_input=False)
    nc.vector.reciprocal(sl(iT), sl(T))
    nc.vector.tensor_tensor(sl(t7), sl(iT), bcol(CAE_C), OP.mult)
    stb = bass.AP(tensor=st.tensor, offset=st.offset,
                  ap=[list(st.ap[0]), [NT, g], [1, NT], [0, 3]])
    nc.vector.tensor_tensor(sl(AE), sl(t7), stb, OP.mult)     # 1/i_kin at b=-1
    nc.vector.memset(sl(b), -1.0)
    # materialize C over the probe axis
    C_b = bass.AP(tensor=C_all.tensor, offset=C_all.offset,
                  ap=[list(C_all.ap[0]), [0, 3], [3 * NT, g], [3, NT], [1, 3]])
    nc.vector.tensor_copy(sl2(Cfull, g), C_b)

    if _DBG_STAGE == 2:
        o = po.work.tile([128, NT, 2], F32, name="dbg2")
        nc.vector.tensor_copy(o[:, :, 0], C_all[:, 0, :, 0])
        nc.vector.tensor_copy(o[:, :, 1], AE[:, 0, :, 0])
        nc.sync.dma_start(out_d.rearrange("(p t) c -> p t c", t=NT), o)
        return

    # ---------- 4-ary climb ----------
    AEb = bass.AP(tensor=AE.tensor, offset=AE.offset,
                  ap=[list(AE.ap[0]), [0, 3], [3 * NT, g], [3, NT], [1, 3]])
    for jj, s in enumerate(STEPS):
        s = float(s)
        AEp = po.work.tile([128, 3, GRPSZ, NT, 3], F32, tag="AEp",
                           name=f"AEp{jj}", bufs=7)
        fstep = bass.AP(tensor=blobsb.tensor, offset=blobsb.offset + BL0 + 9 * jj,
                        ap=[list(blobsb.ap[0]), [3, 3], [0, g], [0, NT], [1, 3]])
        nc.gpsimd.tensor_tensor(sl2(AEp, g), AEb, fstep, OP.mult)
        P = po.work.tile([128, 3, GRPSZ, NT, 3], F32, tag="P", name=f"P{jj}",
                         bufs=7)
        nc.gpsimd.tensor_tensor(sl2(P, g), sl2(AEp, g), sl2(Cfull, g), OP.add)
        S = po.work.tile([128, 3, GRPSZ, NT, 3], F32, tag="S", name=f"S{jj}",
                         bufs=7)
        nc.vector.reciprocal(sl2(S, g), sl2(P, g))
        itot = po.work.tile([128, 3, GRPSZ, NT], F32, tag="it", name=f"it{jj}",
                            bufs=7)
        nc.vector.reduce_sum(sl2(itot, g), sl2(S, g), axis=mybir.AxisListType.X,
                             opt_input=False)
        # cp[p, r, t, j] = (i_tot at probe j >= target), transposed write
        cp = po.work.tile([128, GRPSZ, NT, 3], F32, tag="cp", name=f"cp{jj}",
                          bufs=7)
        cpw = bass.AP(tensor=cp.tensor, offset=cp.offset,
                      ap=[list(cp.ap[0]), [1, 3], [3 * NT, g], [3, NT]])
        nc.vector.tensor_scalar(cpw, sl2(itot, g), I_TARGET, None, OP.is_ge)
        # u = #successful probes; b += s*u; AE *= exp(sc*s*u)
        u = po.work.tile([128, GRPSZ, NT], F32, tag="u", name=f"u{jj}", bufs=7)
        nc.vector.reduce_sum(sl(u), sl(cp), axis=mybir.AxisListType.X,
                             opt_input=False)
        nc.vector.scalar_tensor_tensor(sl(b), sl(u), s, sl(b), OP.mult, OP.add)
        garg = po.work.tile([128, GRPSZ, NT, 3], F32, tag="garg",
                            name=f"garg{jj}", bufs=7)
        ub = bass.AP(tensor=u.tensor, offset=u.offset,
                     ap=[list(u.ap[0]), [NT, g], [1, NT], [0, 3]])
        nc.gpsimd.tensor_tensor(sl(garg), ub, bcol(LNF_C + 3 * jj), OP.mult)
        G = po.work.tile([128, GRPSZ, NT, 3], F32, tag="G", name=f"G{jj}",
                         bufs=7)
        nc.scalar.activation(sl(G), sl(garg), AF.Exp, scale=1.0)
        nc.vector.tensor_tensor(sl(AE), sl(AE), sl(G), OP.mult)

    if _DBG_STAGE == 3:
        nc.sync.dma_start(out_d.rearrange("(p t) c -> p t c", t=NT)[:, :, 0],
                          b[:, 0])
        return

    # ---------- refine: the two bracketing real-grid points ----------
    # g0 = max(b, 0), g1 = b+1 <= 999; only b < 0 needs predication.
    d0, tot, rtot = (w3(n) for n in ("d0", "tot", "rtot"))
    pneg = w3("pneg", I32)
    pick0 = w3("pick0", I32)
    AEf, Ssel = (w33(n) for n in ("AEf", "Ssel"))
    pnegk = po.work.tile([128, GRPSZ, NT, 3], I32, tag="pnegk", name="pnegk")
    pick0k = po.work.tile([128, GRPSZ, NT, 3], I32, tag="pick0k", name="pick0k")
    SP = po.work.tile([128, 2, GRPSZ, NT, 3], F32, tag="SP", name="SP")
    SS = po.work.tile([128, 2, GRPSZ, NT, 3], F32, tag="SS", name="SS")
    it2 = po.work.tile([128, 2, GRPSZ, NT], F32, tag="it2", name="it2")
    fe3 = po.work.tile([128, GRPSZ, NT, 2], F32, tag="fe3", name="fe3")

    nc.vector.tensor_scalar(sl(pneg), sl(b), -0.5, None, OP.is_le)

    def m16(t):  # [p, r, t] broadcast over k
        return bass.AP(tensor=t.tensor, offset=t.offset,
                       ap=[list(t.ap[0]), [NT, g], [1, NT], [0, 3]])

    nc.vector.tensor_copy(sl(pnegk), m16(pneg))
    nc.vector.tensor_tensor(sl(AEf), sl(AE), bcol(F1_C), OP.mult)
    # g0: AE normally; AE*f1 if b<0.  g1: always AE*f1.
    nc.scalar.activation(sl2(SP, g)[:, 0], sl(AE), AF.Copy, scale=1.0)
    nc.vector.copy_predicated(sl2(SP, g)[:, 0], sl(pnegk), sl(AEf))
    nc.scalar.activation(sl2(SP, g)[:, 1], sl(AEf), AF.Copy, scale=1.0)
    nc.gpsimd.tensor_tensor(sl2(SP, g), sl2(SP, g), sl2(Cfull, g)[:, 0:2],
                            OP.add)
    nc.vector.reciprocal(sl2(SS, g), sl2(SP, g))
    nc.vector.reduce_sum(sl2(it2, g), sl2(SS, g), axis=mybir.AxisListType.X,
                         opt_input=False)
    nc.gpsimd.tensor_tensor(sl(d0), sl2(it2, g)[:, 0], sl2(it2, g)[:, 1], OP.add)
    # d0 <= d1  <=>  it0 + it1 <= 2*target (i_tot monotone decreasing)
    nc.vector.tensor_scalar(sl(pick0), sl(d0), 2.0 * I_TARGET, None, OP.is_le)
    nc.vector.tensor_copy(sl(pick0k), m16(pick0))
    nc.vector.tensor_copy(sl(Ssel), sl2(SS, g)[:, 1])
    nc.vector.copy_predicated(sl(Ssel), sl(pick0k), sl2(SS, g)[:, 0])
    nc.vector.reduce_sum(sl(tot), sl(Ssel), axis=mybir.AxisListType.X,
                         opt_input=False)
    nc.vector.reciprocal(sl(rtot), sl(tot))
    nc.gpsimd.tensor_tensor(sl(fe3)[:, :, :, 0], sl(Ssel)[:, :, :, 1], sl(rtot),
                            OP.mult)  # FE_C2H4
    nc.gpsimd.tensor_tensor(sl(fe3)[:, :, :, 1], sl(Ssel)[:, :, :, 0], sl(rtot),
                            OP.mult)  # FE_CO
    for r in range(g):
        nc.sync.dma_start(out_d.rearrange("(p t) c -> p t c", t=NT), fe3[:, r])


def sl2(t, g):
    """restrict a tile with rep axis at dim 2 to g reps"""
    return t[:, :, 0:g] if g != GRPSZ else t


def _build(reps=1):
    from contextlib import ExitStack

    nc = bacc.Bacc("TRN2", target_bir_lowering=False, debug=False)
    xT_d = nc.dram_tensor("xT", [5, NPC], F32R, kind="ExternalInput").ap()
    W1_d = nc.dram_tensor("W1", [5, HID], F32R, kind="ExternalInput").ap()
    pack_d = nc.dram_tensor("pack", [HID, 137], F32R, kind="ExternalInput").ap()
    blobz_d = nc.dram_tensor("blobz", [128, NT + NBLOB + 6], F32,
                             kind="ExternalInput").ap()
    out_d = nc.dram_tensor("out", [NPC, 2], F32, kind="ExternalOutput").ap()
    with tile.TileContext(nc) as tc:
        with ExitStack() as ctx:
            po = _mk_pools(ctx, tc)
            io = (xT_d, W1_d, pack_d, blobz_d, out_d)
            done = 0
            while done < reps:
                g = min(GRPSZ, reps - done)
                lat3 = po.work.tile([128, GRPSZ, NT, 6], F32, tag="lat3",
                                    name="lat3")
                azlt3 = po.work.tile([128, GRPSZ, NT], F32, tag="azlt3",
                                     name="azlt3")
                shared = {}
                gens = [_mlp(tc, po, io, r, g, lat3, azlt3, shared,
                             first=(done == 0 and r == 0)) for r in range(g)]
                while gens:
                    nxt = []
                    for gen in gens:
                        try:
                            next(gen)
                            nxt.append(gen)
                        except StopIteration:
                            pass
                    gens = nxt
                _tail(tc, po, out_d, g, lat3, azlt3, shared[0])
                done += g
    nc.compile()
    return nc


_CACHE = {}


def _make_inputs(x, W1, b1, W2, b2, W3, b3, W4, b4, i0, alpha):
    x = np.ascontiguousarray(np.asarray(x, np.float32))
    pack = np.concatenate(
        [np.asarray(W2, np.float32), np.asarray(W3, np.float32),
         np.asarray(W4, np.float32), np.asarray(b1, np.float32)[:, None],
         np.asarray(b2, np.float32)[:, None], np.asarray(b3, np.float32)[:, None]],
        axis=1)
    blob_row = _make_blob_row(i0, alpha)
    b4f = np.asarray(b4, np.float32)
    in_maps = []
    for c in range(NCORES):
        shard = x[c * NPC:(c + 1) * NPC]
        xT = np.ascontiguousarray(
            shard.reshape(128, NT, 5).transpose(2, 1, 0).reshape(5, NPC))
        blobz = np.empty((128, NT + NBLOB + 6), np.float32)
        blobz[:, 0:NT] = shard[:, 3].reshape(128, NT)
        blobz[:, NT:NT + NBLOB] = blob_row
        blobz[:, NT + NBLOB:] = b4f
        in_maps.append({"xT": xT, "W1": np.ascontiguousarray(W1, np.float32),
                        "pack": np.ascontiguousarray(pack), "blobz": blobz})
    return in_maps


def kernel(x, W1, b1, W2, b2, W3, b3, W4, b4, i0, alpha):
    from concourse.bass_utils import run_bass_kernel_spmd

    if "nc" not in _CACHE:
        _CACHE["nc"] = _build()
    nc = _CACHE["nc"]
    in_maps = _make_inputs(x, W1, b1, W2, b2, W3, b3, W4, b4, i0, alpha)
    res = run_bass_kernel_spmd(nc, in_maps, core_ids=list(range(NCORES)))
    return np.concatenate([res.results[c]["out"] for c in range(NCORES)], axis=0)


# revision 61
# speedup vs baseline: 10.7700x; 3.3204x over previous
"""Trainium2 Bass kernel for the CO2-electrolysis surrogate model.

Contract: kernel(**inputs) takes FULL unsharded inputs (x [16384,5], MLP
weights, kinetic params i0/alpha) and returns the FULL [16384,2] output.
Internally: batch is sharded 2048-per-core across 8 NeuronCores (pure data
parallel).

Design (v4):
- x is transposed on the host into xT [5, 2048] (column c = t*128+p holds
  sample s = p*16+t) so the MLP needs no PE transposes. zlt (= x[:,3]), the
  search/kinetics constant table (functions of i0/alpha), and b4 ride in one
  packed [128, 91] input; 4 input DMAs per rep.
- Matmul inputs are float32r end-to-end (1 cycle/row at N=512 vs 4 for fp32).
- Relu reads two-bank PSUM tiles [128, 2x512], split DVE/ACT (tensor_scalar
  is not implemented on the Pool engine).
- Reps are emitted in groups of G: the MLPs run per rep (interleaved
  emission), then ONE grouped tail runs the per-sample physics for all G
  reps with a leading rep axis in every tile - 3x fewer instructions and
  semaphore hops on the serial search chain.
- The voltage search keeps multiplicative state AE[p,r,t,k] = 1/i_kin at the
  current virtual-grid index b; each 4-ary step probes b+{1,2,3}*s via baked
  factor multiplies and updates AE *= exp(sc*s*u) with u = #successful
  probes (no exp of per-sample args; b stays off the critical path).
- The parameter section uses only {exp, ln, abs, relu, copy}, all inside
  activation-table set 6 (natural_log_exp_and_others), loaded once.
  Identities: 1/(1-sigmoid(l)) = 1+e^l, sigmoid(l)^-1.5 = exp(1.5*ln(1+e^-l)).
- b <= 998 always (i_tot(999) < 1e-3 << target for any theta <= 1), so the
  refine step only needs the b < 0 boundary case.
"""

import sys

for _p in ("/opt/trn_rl_repo", "/opt/pypackages"):
    if _p not in sys.path:
        sys.path.insert(0, _p)

import math

import numpy as np

import concourse.bacc as bacc
import concourse.bass as bass
import concourse.tile as tile
from concourse import mybir

F32 = mybir.dt.float32
F32R = mybir.dt.float32r
I32 = mybir.dt.int32
AF = mybir.ActivationFunctionType
OP = mybir.AluOpType

# ---- problem constants (match reference.py) ----
N = 16384
NCORES = 8
NPC = N // NCORES            # 2048 samples per core
NT = NPC // 128              # 16 tiles of 128 samples
HID = 64
GRID = 1000
VMIN, VMAX = -1.25, 0.0
I_TARGET = 200.0
F_CONST = 96485.33
RT = 8.314 * 298.15
D_CO2 = 1.91e-9
C_CO2 = 34.0
E_EQ = (-0.11, 0.08, 0.0)
N_ELEC_CO2 = (2.0, 12.0)
DV = (VMAX - VMIN) / (GRID - 1)
FRT = F_CONST / RT
STEPS = [256, 64, 16, 4, 1]   # 4-ary climb over virtual 1024-grid, b in [-1,1022]
GRPSZ = 3                     # reps per grouped tail
_DBG_STAGE = 0

# blobz layout: [zlt(16) | blob(NBLOB) | b4(6)]
BL0 = NT                      # blob base column inside blobz
LNF_C = 45
CAE_C = 60
F1_C = 63
LN4_C = 66
CIL_C = 67
NBLOB = 70


def _make_blob_row(i0, alpha):
    """Search/kinetics constants [NBLOB] f32 (functions of i0/alpha)."""
    i0 = np.asarray(i0, np.float64)
    alpha = np.asarray(alpha, np.float64)
    sc = [float(alpha[k] * FRT * DV) for k in range(3)]
    t0 = [float(alpha[k] * FRT * (VMIN - E_EQ[k])) for k in range(3)]
    cols = []
    for s in STEPS:                 # 0:45  probe factors exp(sc*j*s)
        for j in (1, 2, 3):
            for k in range(3):
                cols.append(np.exp(sc[k] * j * s))
    for s in STEPS:                 # 45:60 ln of climb factor: sc_k * s
        for k in range(3):
            cols.append(sc[k] * s)
    for k in range(3):              # 60:63 cAE: exp(t0-sc)/i0 (AE at b=-1, pre 1/theta)
        cols.append(np.exp(t0[k] - sc[k]) / float(i0[k]))
    for k in range(3):              # 63:66 f1: one-step factor exp(sc)
        cols.append(np.exp(sc[k]))
    cols.append(math.log(4e-8))     # 66    bias for the r/Kdl exp
    # 67:70  1/i_lim prefactors (3rd species: H2, not transport-limited -> 0)
    for nk in N_ELEC_CO2:
        cols.append(1.0 / (float(np.float32(np.float32(nk) * F_CONST))
                           * C_CO2 * D_CO2))
    cols.append(0.0)
    row = np.asarray(cols, np.float32)
    assert row.size == NBLOB
    return row


class _Pools:
    pass


def _mk_pools(ctx, tc):
    p = _Pools()
    p.io = ctx.enter_context(tc.tile_pool(name="io", bufs=3))
    p.work = ctx.enter_context(tc.tile_pool(name="work", bufs=3))
    p.psum = ctx.enter_context(tc.tile_pool(name="psum", bufs=3, space="PSUM"))
    return p


def _mlp(tc, po, io, r, g, lat3, azlt3, shared, first):
    """Generator: per-rep DMAs + MLP; writes lat into lat3[:, r] and
    |zlt| into azlt3[:, r].  shared[0] collects the rep's blobsb tile."""
    nc = tc.nc
    xT_d, W1_d, pack_d, blobz_d, out_d = io

    if first:
        # lock the activation table to set 6 (natural_log_exp_and_others):
        # covers exp/ln/abs/relu/copy -> zero reloads for the whole program
        inst = mybir.InstLoadActFuncSet(
            name=nc.get_next_instruction_name(), act_func_set_id=6, ins=[], outs=[])
        nc.scalar.add_instruction(inst)

    xTsb = po.io.tile([5, 4, 512], F32R, name="xTsb")
    nc.sync.dma_start(xTsb, xT_d.rearrange("k (i n) -> k i n", i=4))
    packsb = po.io.tile([64, 137], F32R, name="packsb")
    nc.sync.dma_start(packsb, pack_d)
    W1sb = po.io.tile([5, 64], F32R, name="W1sb")
    nc.sync.dma_start(W1sb, W1_d)
    blobsb = po.io.tile([128, NT + NBLOB + 6], F32, name="blobsb")
    nc.sync.dma_start(blobsb, blobz_d)
    shared[r] = blobsb

    W2sb = packsb[:, 0:64]
    W3sb = packsb[:, 64:128]
    W4sb = packsb[:, 128:134]
    biases = [packsb[:, 134 + i:135 + i].bitcast(F32) for i in range(3)]

    def layer(W, movsrc, bias, name, eng):
        ps = [po.psum.tile([128, 2, 512], F32, tag="ps2", name=f"{name}ps{i}")
              for i in range(2)]
        for i in range(4):
            nc.tensor.matmul(ps[i // 2][0:64, i % 2, :], W, movsrc(i))
        h = po.work.tile([64, 4, 512], F32R, tag=name, name=name, bufs=3)
        for i in range(2):
            dst = h[:, 2 * i:2 * i + 2, :]
            src = ps[i][0:64, :, :]
            if eng[i] == "v":
                nc.vector.tensor_scalar(dst, src, bias, 0.0, OP.add, OP.max)
            else:
                nc.scalar.activation(dst, src, AF.Relu, bias=bias, scale=1.0)
        return h

    yield
    h1 = layer(W1sb, lambda i: xTsb[:, i, :], biases[0], "h1", "av")
    nc.scalar.activation(azlt3[:, r], blobsb[:, 0:NT], AF.Abs, scale=1.0)
    yield
    h2 = layer(W2sb, lambda i: h1[:, i, :], biases[1], "h2", "va")
    yield
    h3 = layer(W3sb, lambda i: h2[:, i, :], biases[2], "h3", "aa")
    yield
    h3f = h3.rearrange("p a c -> p (a c)")

    latps = po.psum.tile([128, 96], F32, tag="lat", name="latps", bufs=2)
    for t in range(NT):
        nc.tensor.matmul(latps[:, t * 6:(t + 1) * 6],
                         h3f[:, t * 128:(t + 1) * 128], W4sb)
    b4b = bass.AP(tensor=blobsb.tensor, offset=blobsb.offset + BL0 + NBLOB,
                  ap=[list(blobsb.ap[0]), [0, NT], [1, 6]])
    nc.vector.tensor_tensor(lat3[:, r], latps.rearrange("p (t j) -> p t j", j=6),
                            b4b, OP.add)


def _tail(tc, po, out_d, g, lat3, azlt3, blobsb):
    """Grouped per-sample physics for g reps: parameters, 4-ary climb,
    refine, FE output.  All tiles carry a leading rep axis of size g."""
    nc = tc.nc
    GNT = g * NT

    def bcol(c, n=3):
        """blob columns c..c+n as [p, r(bcast), t(bcast), k] AP"""
        return bass.AP(tensor=blobsb.tensor, offset=blobsb.offset + BL0 + c,
                       ap=[list(blobsb.ap[0]), [0, g * NT], [1, n]])

    if _DBG_STAGE == 1:
        nc.sync.dma_start(out_d.rearrange("(p t) c -> p t c", t=NT),
                          lat3[:, 0, :, 0:2])
        return

    def w3(name, dt=F32):
        return po.work.tile([128, g, NT], dt, tag=name, name=name)

    def w33(name):
        return po.work.tile([128, g, NT, 3], F32, tag=name, name=name)

    a1, e1i, a2, a3, d1, a4, Lt, s5, t6, mm, st, b = (
        w3(n) for n in ("a1", "e1i", "a2", "a3", "d1", "a4", "Lt",
                        "s5", "t6", "mm", "st", "b"))
    C_all, dk, T, iT, t7, AE = (
        w33(n) for n in ("C_all", "dk", "T", "iT", "t7", "AE"))
    Cfull = po.work.tile([128, 3, g, NT, 3], F32, tag="Cfull", name="Cfull")

    def sl(t):
        return t

    l3 = lat3
    nc.scalar.activation(sl(a1), l3[:, :, :, 1], AF.Exp, scale=-1.0)   # e^-l1
    nc.vector.reciprocal(sl(e1i), sl(a1))                              # e^l1
    nc.scalar.activation(sl(a2), sl(a1), AF.Ln, bias=1.0, scale=1.0)   # ln(1+e^-l1)
    nc.scalar.activation(sl(a3), sl(a2), AF.Exp, scale=1.5)            # eps^-1.5
    nc.gpsimd.tensor_tensor(sl(d1), l3[:, :, :, 0], l3[:, :, :, 2], OP.subtract)
    nc.scalar.activation(sl(a4), sl(d1), AF.Exp,
                         bias=blobsb[:, BL0 + LN4_C:BL0 + LN4_C + 1],
                         scale=1.0)                                    # r/Kdl
    nc.vector.scalar_tensor_tensor(sl(Lt), sl(e1i), 1.0, sl(azlt3),
                                   OP.add, OP.mult)
    nc.gpsimd.tensor_tensor(sl(s5), sl(a4), sl(Lt), OP.add)
    nc.gpsimd.tensor_tensor(sl(t6), sl(s5), sl(a3), OP.mult)
    # C_all[:, r, :, k] = t6 * cilim_k  (k=2 blob column is 0 -> H2 term)
    t6b = bass.AP(tensor=t6.tensor, offset=t6.offset,
                  ap=[list(t6.ap[0]), [1, g * NT], [0, 3]])
    nc.vector.tensor_tensor(sl(C_all), t6b, bcol(CIL_C), OP.mult)

    # softmax thetas -> AE init at b=-1
    nc.vector.reduce_max(sl(mm), l3[:, :, :, 3:6], axis=mybir.AxisListType.X,
                         opt_input=False)
    mmb = bass.AP(tensor=mm.tensor, offset=mm.offset,
                  ap=[list(mm.ap[0]), [1, g * NT], [0, 3]])
    nc.vector.tensor_tensor(sl(dk), l3[:, :, :, 3:6], mmb, OP.subtract)
    nc.scalar.activation(sl(T), sl(dk), AF.Exp, scale=2.0)
    nc.vector.reduce_sum(sl(st), sl(T), axis=mybir.AxisListType.X,
                         opt_input=False)
    nc.vector.reciprocal(sl(iT), sl(T))
    yield
    nc.vector.tensor_tensor(sl(t7), sl(iT), bcol(CAE_C), OP.mult)
    stb = bass.AP(tensor=st.tensor, offset=st.offset,
                  ap=[list(st.ap[0]), [1, g * NT], [0, 3]])
    nc.vector.tensor_tensor(sl(AE), sl(t7), stb, OP.mult)     # 1/i_kin at b=-1
    nc.vector.memset(sl(b), -1.0)
    # materialize C over the probe axis
    C_b = bass.AP(tensor=C_all.tensor, offset=C_all.offset,
                  ap=[list(C_all.ap[0]), [0, 3], [1, g * NT * 3]])
    nc.vector.tensor_copy(Cfull, C_b)
    yield

    if _DBG_STAGE == 2:
        o = po.work.tile([128, NT, 2], F32, name="dbg2")
        nc.vector.tensor_copy(o[:, :, 0], C_all[:, 0, :, 0])
        nc.vector.tensor_copy(o[:, :, 1], AE[:, 0, :, 0])
        nc.sync.dma_start(out_d.rearrange("(p t) c -> p t c", t=NT), o)
        return

    # ---------- 4-ary climb ----------
    AEb = bass.AP(tensor=AE.tensor, offset=AE.offset,
                  ap=[list(AE.ap[0]), [0, 3], [1, g * NT * 3]])
    for jj, s in enumerate(STEPS):
        s = float(s)
        AEp = po.work.tile([128, 3, g, NT, 3], F32, tag="AEp",
                           name=f"AEp{jj}", bufs=7)
        fstep = bass.AP(tensor=blobsb.tensor, offset=blobsb.offset + BL0 + 9 * jj,
                        ap=[list(blobsb.ap[0]), [3, 3], [0, g * NT], [1, 3]])
        nc.gpsimd.tensor_tensor(AEp, AEb, fstep, OP.mult)
        P = po.work.tile([128, 3, g, NT, 3], F32, tag="P", name=f"P{jj}",
                         bufs=7)
        nc.gpsimd.tensor_tensor(P, AEp, Cfull, OP.add)
        S = po.work.tile([128, 3, g, NT, 3], F32, tag="S", name=f"S{jj}",
                         bufs=7)
        nc.vector.reciprocal(S, P)
        itot = po.work.tile([128, 3, g, NT], F32, tag="it", name=f"it{jj}",
                            bufs=7)
        nc.vector.reduce_sum(itot, S, axis=mybir.AxisListType.X,
                             opt_input=False)
        yield
        # cp[p, r, t, j] = (i_tot at probe j >= target), transposed write
        cp = po.work.tile([128, g, NT, 3], F32, tag="cp", name=f"cp{jj}",
                          bufs=7)
        cpw = bass.AP(tensor=cp.tensor, offset=cp.offset,
                      ap=[list(cp.ap[0]), [1, 3], [3, g * NT]])
        nc.vector.tensor_scalar(cpw, itot, I_TARGET, None, OP.is_ge)
        # u = #successful probes; b += s*u; AE *= exp(sc*s*u)
        u = po.work.tile([128, g, NT], F32, tag="u", name=f"u{jj}", bufs=7)
        nc.vector.reduce_sum(sl(u), sl(cp), axis=mybir.AxisListType.X,
                             opt_input=False)
        nc.vector.scalar_tensor_tensor(sl(b), sl(u), s, sl(b), OP.mult, OP.add)
        garg = po.work.tile([128, g, NT, 3], F32, tag="garg",
                            name=f"garg{jj}", bufs=7)
        ub = bass.AP(tensor=u.tensor, offset=u.offset,
                     ap=[list(u.ap[0]), [1, g * NT], [0, 3]])
        nc.gpsimd.tensor_tensor(sl(garg), ub, bcol(LNF_C + 3 * jj), OP.mult)
        G = po.work.tile([128, g, NT, 3], F32, tag="G", name=f"G{jj}",
                         bufs=7)
        nc.scalar.activation(sl(G), sl(garg), AF.Exp, scale=1.0)
        nc.gpsimd.tensor_tensor(sl(AE), sl(AE), sl(G), OP.mult)
        yield

    if _DBG_STAGE == 3:
        nc.sync.dma_start(out_d.rearrange("(p t) c -> p t c", t=NT)[:, :, 0],
                          b[:, 0])
        return

    # ---------- refine: the two bracketing real-grid points ----------
    # g0 = max(b, 0), g1 = b+1 <= 999; only b < 0 needs predication.
    d0, tot, rtot = (w3(n) for n in ("d0", "tot", "rtot"))
    pneg = w3("pneg", I32)
    pick0 = w3("pick0", I32)
    AEf, Ssel = (w33(n) for n in ("AEf", "Ssel"))
    pnegk = po.work.tile([128, g, NT, 3], I32, tag="pnegk", name="pnegk")
    pick0k = po.work.tile([128, g, NT, 3], I32, tag="pick0k", name="pick0k")
    SP = po.work.tile([128, 2, g, NT, 3], F32, tag="SP", name="SP")
    SS = po.work.tile([128, 2, g, NT, 3], F32, tag="SS", name="SS")
    it2 = po.work.tile([128, 2, g, NT], F32, tag="it2", name="it2")
    fe3 = po.work.tile([128, g, NT, 2], F32, tag="fe3", name="fe3")

    nc.vector.tensor_scalar(sl(pneg), sl(b), -0.5, None, OP.is_le)

    def m16(t):  # [p, r, t] broadcast over k
        return bass.AP(tensor=t.tensor, offset=t.offset,
                       ap=[list(t.ap[0]), [1, g * NT], [0, 3]])

    nc.vector.tensor_copy(sl(pnegk), m16(pneg))
    nc.vector.tensor_tensor(sl(AEf), sl(AE), bcol(F1_C), OP.mult)
    # g0: AE normally; AE*f1 if b<0.  g1: always AE*f1.
    nc.scalar.activation(SP[:, 0], sl(AE), AF.Copy, scale=1.0)
    nc.vector.copy_predicated(SP[:, 0], sl(pnegk), sl(AEf))
    nc.scalar.activation(SP[:, 1], sl(AEf), AF.Copy, scale=1.0)
    yield
    nc.gpsimd.tensor_tensor(SP, SP, Cfull[:, 0:2],
                            OP.add)
    nc.vector.reciprocal(SS, SP)
    nc.vector.reduce_sum(it2, SS, axis=mybir.AxisListType.X,
                         opt_input=False)
    nc.gpsimd.tensor_tensor(sl(d0), it2[:, 0], it2[:, 1], OP.add)
    # d0 <= d1  <=>  it0 + it1 <= 2*target (i_tot monotone decreasing)
    nc.vector.tensor_scalar(sl(pick0), sl(d0), 2.0 * I_TARGET, None, OP.is_le)
    nc.vector.tensor_copy(sl(pick0k), m16(pick0))
    nc.vector.tensor_copy(sl(Ssel), SS[:, 1])
    nc.vector.copy_predicated(sl(Ssel), sl(pick0k), SS[:, 0])
    nc.vector.reduce_sum(sl(tot), sl(Ssel), axis=mybir.AxisListType.X,
                         opt_input=False)
    nc.vector.reciprocal(sl(rtot), sl(tot))
    nc.gpsimd.tensor_tensor(sl(fe3)[:, :, :, 0], sl(Ssel)[:, :, :, 1], sl(rtot),
                            OP.mult)  # FE_C2H4
    nc.gpsimd.tensor_tensor(sl(fe3)[:, :, :, 1], sl(Ssel)[:, :, :, 0], sl(rtot),
                            OP.mult)  # FE_CO
    for r in range(g):
        nc.sync.dma_start(out_d.rearrange("(p t) c -> p t c", t=NT), fe3[:, r])





def _build(reps=1):
    from contextlib import ExitStack

    nc = bacc.Bacc("TRN2", target_bir_lowering=False, debug=False)
    xT_d = nc.dram_tensor("xT", [5, NPC], F32R, kind="ExternalInput").ap()
    W1_d = nc.dram_tensor("W1", [5, HID], F32R, kind="ExternalInput").ap()
    pack_d = nc.dram_tensor("pack", [HID, 137], F32R, kind="ExternalInput").ap()
    blobz_d = nc.dram_tensor("blobz", [128, NT + NBLOB + 6], F32,
                             kind="ExternalInput").ap()
    out_d = nc.dram_tensor("out", [NPC, 2], F32, kind="ExternalOutput").ap()
    def drive(gens):
        while gens:
            nxt = []
            for gen in gens:
                try:
                    next(gen)
                    nxt.append(gen)
                except StopIteration:
                    pass
            gens = nxt

    with tile.TileContext(nc) as tc:
        with ExitStack() as ctx:
            po = _mk_pools(ctx, tc)
            io = (xT_d, W1_d, pack_d, blobz_d, out_d)
            # group sizes, processed in pairs of groups whose tails interleave
            sizes = []
            left = reps
            while left > 0:
                sizes.append(min(GRPSZ, left))
                left -= sizes[-1]
            done = 0
            for p0 in range(0, len(sizes), 2):
                tails = []
                for g in sizes[p0:p0 + 2]:
                    lat3 = po.work.tile([128, g, NT, 6], F32, tag="lat3",
                                        name="lat3")
                    azlt3 = po.work.tile([128, g, NT], F32, tag="azlt3",
                                         name="azlt3")
                    shared = {}
                    drive([_mlp(tc, po, io, r, g, lat3, azlt3, shared,
                                first=(done == 0 and r == 0))
                           for r in range(g)])
                    tails.append(_tail(tc, po, out_d, g, lat3, azlt3, shared[0]))
                    done += g
                drive(tails)
    nc.compile()
    return nc


_CACHE = {}


def _make_inputs(x, W1, b1, W2, b2, W3, b3, W4, b4, i0, alpha):
    x = np.ascontiguousarray(np.asarray(x, np.float32))
    pack = np.concatenate(
        [np.asarray(W2, np.float32), np.asarray(W3, np.float32),
         np.asarray(W4, np.float32), np.asarray(b1, np.float32)[:, None],
         np.asarray(b2, np.float32)[:, None], np.asarray(b3, np.float32)[:, None]],
        axis=1)
    blob_row = _make_blob_row(i0, alpha)
    b4f = np.asarray(b4, np.float32)
    in_maps = []
    for c in range(NCORES):
        shard = x[c * NPC:(c + 1) * NPC]
        xT = np.ascontiguousarray(
            shard.reshape(128, NT, 5).transpose(2, 1, 0).reshape(5, NPC))
        blobz = np.empty((128, NT + NBLOB + 6), np.float32)
        blobz[:, 0:NT] = shard[:, 3].reshape(128, NT)
        blobz[:, NT:NT + NBLOB] = blob_row
        blobz[:, NT + NBLOB:] = b4f
        in_maps.append({"xT": xT, "W1": np.ascontiguousarray(W1, np.float32),
                        "pack": np.ascontiguousarray(pack), "blobz": blobz})
    return in_maps


def kernel(x, W1, b1, W2, b2, W3, b3, W4, b4, i0, alpha):
    from concourse.bass_utils import run_bass_kernel_spmd

    if "nc" not in _CACHE:
        _CACHE["nc"] = _build()
    nc = _CACHE["nc"]
    in_maps = _make_inputs(x, W1, b1, W2, b2, W3, b3, W4, b4, i0, alpha)
    res = run_bass_kernel_spmd(nc, in_maps, core_ids=list(range(NCORES)))
    return np.concatenate([res.results[c]["out"] for c in range(NCORES)], axis=0)
